# revision 1
# baseline (speedup 1.0000x reference)
"""Trainium2 Bass kernel: 3x3 single-channel conv (stride 1, pad 1) on a
4096x4096 fp32 image, sharded over 8 NeuronCores by rows of H.

Numerics: x and w are split on host into fp16 head+tail (x = xh + xl,
w = wh + wl to ~2^-22 relative); conv(x, w) is computed on TensorE as
  S(wh) @ xh + S(wl) @ xh + S(wh) @ xl
(error ~ wl*xl ~ 2^-22), where S(.) is a banded lhsT [128, 128] encoding
the three vertical taps and the horizontal taps come from dj free-dim
offsets of the rhs access pattern. fp16 matmuls run at 1 cyc/row vs 4
for fp32. 9 matmuls accumulate per PSUM chunk.

Per core (512 output rows): 4 full tiles of 126 rows + an 8-row tail
computed with 8 column-groups stacked in the partition dim (streams
512 cols through the PE instead of 4096).
"""
import sys
sys.path.insert(0, '/opt/trn_rl_repo')
import numpy as np

import concourse.bass as bass
import concourse.mybir as mybir
from concourse.tile import TileContext
from concourse import bass_utils

H = W = 4096
N_CORES = 8
ROWS_PER_CORE = H // N_CORES          # 512
TILE_OUT = 126                        # clean output rows per 128-row tile
CHUNK = 512                           # matmul moving free dim (one PSUM bank)
N_CHUNKS = W // CHUNK                 # 8
FULL_TILES = ROWS_PER_CORE // TILE_OUT        # 4
TAIL_ROWS = ROWS_PER_CORE - FULL_TILES * TILE_OUT   # 8
WPAD = W + 2                          # 4098
TAIL_G = 8                            # tail column groups
TAIL_GW = W // TAIL_G                 # 1024
TAIL_K = TAIL_ROWS + 2                # 10 rows per group
TAIL_STACK = TAIL_G * TAIL_K          # 40 partitions
TAIL_M = TAIL_G * TAIL_ROWS           # 32 psum rows

_cache = {}


def _split_multi_waits(nc):
    """This container's walrus accepts only one sync-wait per instruction;
    Tile's tail drain can carry several. Split extras onto NOPs."""
    ctr = 0
    for f in nc.m.functions:
        for bb in f.blocks:
            new_insts = []
            for ins in bb.instructions:
                si = ins.sync_info
                if si is not None and si.on_wait and len(si.on_wait) > 1:
                    waits = list(si.on_wait)
                    for wt in waits[:-1]:
                        ctr += 1
                        new_insts.append(mybir.InstNoOp(
                            name=f"waitfix_{ctr}",
                            sync_info=mybir.SyncInfo(on_wait=[wt], on_update=[]),
                            bass_nofuse=True,
                            engine=ins.engine,
                        ))
                    si.on_wait = [waits[-1]]
                new_insts.append(ins)
            bb.instructions[:] = new_insts
    return nc


def _build_nc(reps=1, mode="full", out_ring="scalar", order="chunk",
              xbounds=(0, 514, 2050, WPAD), xbufs=3, osplit=2, hint=True,
              psum_bufs=4, copy_eng="act", unroll=1, obufs=2, alt_rings=False,
              tail_pos=99, chunk=CHUNK, dma_prio=None, out_prio=None,
              merge_tail_out=False, last_osplit=4, split_ot=False):
    f32 = mybir.dt.float32
    f16 = mybir.dt.float16
    do_pe = mode in ("full", "pe_only")
    do_act = mode == "full"
    do_out = mode in ("full", "dma_only")
    nc = bass.Bass()
    # hi/lo packed per row: [rows, 2, WPAD] fp16 (dim 1: 0=hi, 1=lo)
    xx_d = nc.dram_tensor("xx", [ROWS_PER_CORE + 2, 2, WPAD], f16,
                          kind="ExternalInput")
    # 3 products x 3 dj, each lhsT [128, 128] (2 zero cols of padding)
    sm_d = nc.dram_tensor("smat", [128, 9 * 128], f16, kind="ExternalInput")
    # tail: 3 products x 3 dj stacked block-diag lhsT [80, 64]
    st_d = nc.dram_tensor("stail", [TAIL_STACK, 9 * TAIL_M], f16,
                          kind="ExternalInput")
    bias_in = nc.dram_tensor("bias_in", [128, 1], f32, kind="ExternalInput")
    y = nc.dram_tensor("y", [ROWS_PER_CORE, W], f32, kind="ExternalOutput")

    with TileContext(nc) as tc:
        with tc.tile_pool(name="consts", bufs=1) as cpool, \
             tc.tile_pool(name="xt", bufs=xbufs) as xpool, \
             tc.tile_pool(name="ot", bufs=obufs) as opool, \
             tc.tile_pool(name="psum",
                          bufs=(psum_bufs if chunk == 512 else 3),
                          space="PSUM") as ppool, \
             tc.tile_pool(name="psumt",
                          bufs=(8 - psum_bufs if chunk == 512 else 2),
                          space="PSUM") as ppool_t:
            # const loads ride the SWDGE (gpsimd) ring so they never queue
            # ahead of tile 0's input pieces on the SP HWDGE FIFO
            s_t = cpool.tile([128, 9 * 128], f16)
            nc.gpsimd.dma_start(s_t[:], sm_d[:])
            st_t = cpool.tile([TAIL_STACK, 9 * TAIL_M], f16)
            nc.gpsimd.dma_start(st_t[:], st_d[:])
            b_t = cpool.tile([128, 1], f32)
            nc.gpsimd.dma_start(b_t[:], bias_in[:])
            zt = None
            if mode == "dma_only":
                zt = cpool.tile([128, W], f32)
                nc.gpsimd.memset(zt[:], 0.0)

            out_eng = nc.scalar if out_ring == "scalar" else nc.sync
            PASSES = ((0, 0), (0, 1), (0, 2),   # (smi, dj) for src xh
                      (1, 0), (1, 1), (1, 2),   # smi=1 -> S_lo on xh
                      (2, 0), (2, 1), (2, 2))   # smi=2 -> S_hi on xl

            def mm_passes(ps_list, srcs, lhs_tile, mwidth, chunk_ids, base_off,
                          cw=CHUNK):
                """Issue 9 passes over the given chunks; pass-outer so
                consecutive matmuls reuse one stationary matrix."""
                npass = len(PASSES)
                if order == "group4":
                    for p, (smi, dj) in enumerate(PASSES):
                        srctile = (srcs[0], srcs[0], srcs[1])[smi]
                        scol = ((smi if smi < 2 else 0) * 3 + dj) * mwidth
                        for ci, c0 in enumerate(chunk_ids):
                            nc.tensor.matmul(
                                ps_list[ci],
                                lhs_tile[:, scol:scol + mwidth],
                                srctile[:, base_off + c0 + dj:
                                        base_off + c0 + dj + cw],
                                start=(p == 0), stop=(p == npass - 1),
                            )
                else:
                    for ci, c0 in enumerate(chunk_ids):
                        for p, (smi, dj) in enumerate(PASSES):
                            srctile = (srcs[0], srcs[0], srcs[1])[smi]
                            scol = ((smi if smi < 2 else 0) * 3 + dj) * mwidth
                            nc.tensor.matmul(
                                ps_list[ci],
                                lhs_tile[:, scol:scol + mwidth],
                                srctile[:, base_off + c0 + dj:
                                        base_off + c0 + dj + cw],
                                start=(p == 0), stop=(p == npass - 1),
                            )

            def full_tile(t):
                k = 128
                r0 = t * TILE_OUT
                xx = xpool.tile([128, 2, WPAD], f16, tag="xx")
                in_eng = (nc.sync, nc.scalar)[t % 2] if alt_rings else nc.sync
                for i in range(len(xbounds) - 1):
                    lo, hi = xbounds[i], xbounds[i + 1]
                    dd = in_eng.dma_start(xx[:k, :, lo:hi],
                                          xx_d[r0:r0 + k, :, lo:hi])
                    if dma_prio is not None:
                        dd.ins.bass_priority = dma_prio
                xh = xx[:, 0, :]
                xl = xx[:, 1, :]
                if split_ot:
                    ot_a = opool.tile([128, W // 2], f32, tag="ota")
                    ot_b = opool.tile([128, W // 2], f32, tag="otb")
                else:
                    ot = opool.tile([128, W], f32, tag="ot")
                n_chunks = W // chunk
                gsz = max(1, (4 * 512) // chunk)   # chunks per 4-bank group
                for g in range(n_chunks // gsz):
                    chunk_ids = [(g * gsz + i) * chunk for i in range(gsz)]
                    if do_pe:
                        ps_list = []
                        for _ in range(gsz):
                            ps_i = ppool.tile([128, chunk], f32, tag="ps")
                            ps_list.append(ps_i[:, :])
                        mm_passes(ps_list, (xh, xl), s_t, 128, chunk_ids, 0,
                                  cw=chunk)
                    if do_act:
                        for ci, c0 in enumerate(chunk_ids):
                            if split_ot:
                                dst_t = ot_a if c0 < W // 2 else ot_b
                                cc = c0 % (W // 2)
                            else:
                                dst_t, cc = ot, c0
                            if copy_eng == "act":
                                nc.scalar.activation(
                                    dst_t[:TILE_OUT, cc:cc + chunk],
                                    ps_list[ci][:TILE_OUT, :],
                                    mybir.ActivationFunctionType.Identity,
                                    bias=b_t[:TILE_OUT, :], scale=1.0,
                                )
                            else:
                                nc.vector.tensor_scalar_add(
                                    dst_t[:TILE_OUT, cc:cc + chunk],
                                    ps_list[ci][:TILE_OUT, :],
                                    b_t[:TILE_OUT, :])
                if do_out:
                    oeng = (nc.scalar, nc.sync)[t % 2] if alt_rings else out_eng
                    if split_ot and do_act:
                        hw2 = W // 2
                        for i, ht in ((0, ot_a), (1, ot_b)):
                            oeng.dma_start(
                                y[r0:r0 + TILE_OUT, i * hw2:(i + 1) * hw2],
                                ht[:TILE_OUT, :])
                    else:
                        src_t = ot if do_act else zt
                        osp = last_osplit if t == FULL_TILES - 1 else osplit
                        ow = W // osp
                        for i in range(osp):
                            od = oeng.dma_start(
                                y[r0:r0 + TILE_OUT, i * ow:(i + 1) * ow],
                                src_t[:TILE_OUT, i * ow:(i + 1) * ow])
                            if out_prio is not None:
                                od.ins.bass_priority = out_prio

            def tail_load():
                r0 = FULL_TILES * TILE_OUT   # shard row 504
                xxs = xpool.tile([TAIL_STACK, 2, TAIL_GW + 2], f16, tag="txx")
                for g in range(TAIL_G):
                    gc = g * TAIL_GW
                    nc.scalar.dma_start(
                        xxs[g * TAIL_K:(g + 1) * TAIL_K, :, :],
                        xx_d[r0:r0 + TAIL_K, :, gc:gc + TAIL_GW + 2])
                return xxs[:, 0, :], xxs[:, 1, :]

            def tail_tile(xh, xl):
                r0 = FULL_TILES * TILE_OUT   # shard row 504
                ot = opool.tile([TAIL_M, TAIL_GW], f32, tag="tot")
                chunk_ids = [c * CHUNK for c in range(TAIL_GW // CHUNK)]
                if do_pe:
                    ps_list = []
                    for _ in range(len(chunk_ids)):
                        ps_i = ppool_t.tile([TAIL_M, CHUNK], f32, tag="tps")
                        ps_list.append(ps_i[:, :])
                    mm_passes(ps_list, (xh, xl), st_t, TAIL_M, chunk_ids, 0)
                if do_act:
                    for ci, c0 in enumerate(chunk_ids):
                        if copy_eng == "act":
                            nc.scalar.activation(
                                ot[:, c0:c0 + CHUNK], ps_list[ci],
                                mybir.ActivationFunctionType.Identity,
                                bias=b_t[:TAIL_M, :], scale=1.0,
                            )
                        else:
                            nc.vector.tensor_scalar_add(
                                ot[:, c0:c0 + CHUNK], ps_list[ci],
                                b_t[:TAIL_M, :])
                if do_out:
                    src_t = ot if do_act else zt
                    if merge_tail_out:
                        dst = y[r0:r0 + TAIL_ROWS, :].rearrange(
                            "r (g c) -> g r c", g=TAIL_G)
                        msrc = src_t[:TAIL_M, :TAIL_GW].rearrange(
                            "(g r) c -> g r c", g=TAIL_G)
                        out_eng.dma_start(dst, msrc)
                    else:
                        for g in range(TAIL_G):
                            out_eng.dma_start(
                                y[r0:r0 + TAIL_ROWS,
                                  g * TAIL_GW:(g + 1) * TAIL_GW],
                                src_t[g * TAIL_ROWS:(g + 1) * TAIL_ROWS,
                                      :TAIL_GW])

            def body():
                txh, txl = tail_load()
                if tail_pos == 0:
                    tail_tile(txh, txl)
                for t in range(FULL_TILES):
                    full_tile(t)
                    if t + 1 == tail_pos:
                        tail_tile(txh, txl)
                if tail_pos > FULL_TILES:
                    tail_tile(txh, txl)

            if reps == 1:
                body()
            else:
                hints = (mybir.EngineType.PE,) if hint else ()
                with tc.For_i(0, reps, 1, hint_engines=hints):
                    for _ in range(unroll):
                        body()

    _split_multi_waits(nc)
    return nc


def _band(w3x3, k, m, row_off=0, col_off=0, out=None, dtype=np.float16):
    """S[row_off + m_ + di, col_off + m_] = w3x3[di, dj-column-block]."""
    if out is None:
        out = np.zeros((k, m), dtype=dtype)
    return out


def _make_smat(wh, wl):
    """[128, 9*128] fp16: product-major blocks (smi*3+dj)*128, each a banded
    lhsT [128, 128] with band weights w[di, dj]; cols 126, 127 are zero."""
    out = np.zeros((128, 9 * 128), dtype=np.float16)
    idx = np.arange(TILE_OUT)
    for smi, wmat in ((0, wh), (1, wl)):
        for dj in range(3):
            blk = out[:, (smi * 3 + dj) * 128:(smi * 3 + dj) * 128 + 128]
            for di in range(3):
                blk[idx + di, idx] = wmat[di, dj]
    return out


def _make_stail(wh, wl):
    """[40, 9*32] fp16: block-diagonal stacked tail lhsT per product/dj."""
    out = np.zeros((TAIL_STACK, 9 * TAIL_M), dtype=np.float16)
    idx = np.arange(TAIL_ROWS)
    for smi, wmat in ((0, wh), (1, wl)):
        for dj in range(3):
            blk = out[:, (smi * 3 + dj) * TAIL_M:(smi * 3 + dj + 1) * TAIL_M]
            for g in range(TAIL_G):
                sub = blk[g * TAIL_K:(g + 1) * TAIL_K,
                          g * TAIL_ROWS:(g + 1) * TAIL_ROWS]
                for di in range(3):
                    sub[idx + di, idx] = wmat[di, dj]
    return out


def kernel(x, weight, bias):
    x = np.asarray(x, dtype=np.float32)
    weight = np.asarray(weight, dtype=np.float32)
    bias = np.asarray(bias, dtype=np.float32)
    w3 = weight.reshape(3, 3)
    wh = w3.astype(np.float16)
    wl = (w3 - wh.astype(np.float32)).astype(np.float16)

    if "nc" not in _cache:
        _cache["nc"] = _build_nc()
    nc = _cache["nc"]

    xh = x.astype(np.float16)
    xl = (x - xh.astype(np.float32)).astype(np.float16)
    xxp = np.zeros((H + 2, 2, WPAD), dtype=np.float16)
    xxp[1:H + 1, 0, 1:W + 1] = xh
    xxp[1:H + 1, 1, 1:W + 1] = xl

    smat = _make_smat(wh, wl)
    stail = _make_stail(wh, wl)
    bias_bc = np.full((128, 1), bias[0], dtype=np.float32)

    in_maps = []
    for c in range(N_CORES):
        r0 = c * ROWS_PER_CORE
        in_maps.append({
            "xx": np.ascontiguousarray(xxp[r0:r0 + ROWS_PER_CORE + 2, :, :]),
            "smat": smat,
            "stail": stail,
            "bias_in": bias_bc,
        })

    _cache["in_maps"] = in_maps
    res = None
    for attempt in range(3):
        try:
            res = bass_utils.run_bass_kernel_spmd(
                nc, in_maps, core_ids=list(range(N_CORES)))
            break
        except Exception:
            if attempt == 2:
                raise
    out = np.empty((H, W), dtype=np.float32)
    for c in range(N_CORES):
        out[c * ROWS_PER_CORE:(c + 1) * ROWS_PER_CORE, :] = res.results[c]["y"]
    return out



# revision 4
# speedup vs baseline: 3.1026x; 3.1026x over previous
"""Trainium2 Bass kernel: 3x3 single-channel conv (stride 1, pad 1) on a
4096x4096 fp32 image, sharded over 8 NeuronCores by rows of H.

Numerics: correctness gate is rel_err < 2e-2 vs the fp32 reference, so x
and w are cast to fp16 and the conv runs as a single fp16 product on
TensorE (error ~5e-4 rel): S(w) @ x where S is a banded lhsT [128, 128]
encoding the three vertical taps; the horizontal taps come from dj
free-dim offsets of the rhs access pattern. 3 matmuls accumulate per
PSUM chunk (vs 9 for the hi/lo split), so PE time is ~22us/core.
Output is written fp16 (quant err ~3e-3 abs on a ~11 max-abs signal) and
upcast to fp32 on host, halving write traffic.

Per core (512 output rows): 4 full tiles of 126 rows + an 8-row tail
computed with 8 column-groups stacked in the partition dim. Engine plan:
input DMA on SP ring, output DMA on the gpsimd SWDGE ring, PSUM->SBUF
bias-add copies alternate between Act (scalar) and DVE (vector).
"""
import sys
sys.path.insert(0, '/opt/trn_rl_repo')
import numpy as np

import concourse.bass as bass
import concourse.mybir as mybir
from concourse.tile import TileContext
from concourse import bass_utils

H = W = 4096
N_CORES = 8
ROWS_PER_CORE = H // N_CORES          # 512
TILE_OUT = 126                        # clean output rows per 128-row tile
CHUNK = 512                           # matmul moving free dim (one PSUM bank)
N_CHUNKS = W // CHUNK                 # 8
FULL_TILES = ROWS_PER_CORE // TILE_OUT        # 4
TAIL_ROWS = ROWS_PER_CORE - FULL_TILES * TILE_OUT   # 8
WPAD = W + 2                          # 4098
TAIL_G = 8                            # tail column groups
TAIL_GW = W // TAIL_G                 # 512
TAIL_K = TAIL_ROWS + 2                # 10 rows per group
TAIL_STACK = TAIL_G * TAIL_K          # 80 partitions
TAIL_M = TAIL_G * TAIL_ROWS           # 64 psum rows

_cache = {}


def _split_multi_waits(nc):
    """This container's walrus accepts only one sync-wait per instruction;
    Tile's tail drain can carry several. Split extras onto NOPs."""
    ctr = 0
    for f in nc.m.functions:
        for bb in f.blocks:
            new_insts = []
            for ins in bb.instructions:
                si = ins.sync_info
                if si is not None and si.on_wait and len(si.on_wait) > 1:
                    waits = list(si.on_wait)
                    for wt in waits[:-1]:
                        ctr += 1
                        new_insts.append(mybir.InstNoOp(
                            name=f"waitfix_{ctr}",
                            sync_info=mybir.SyncInfo(on_wait=[wt], on_update=[]),
                            bass_nofuse=True,
                            engine=ins.engine,
                        ))
                    si.on_wait = [waits[-1]]
                new_insts.append(ins)
            bb.instructions[:] = new_insts
    return nc


def _build_nc(reps=1, mode="full", out_ring="scalar", in_ring="sync",
              xbounds=(0, 514, 2050, WPAD), xbufs=3, osplit=2, hint=True,
              psum_bufs=6, copy_split="alt", unroll=1, obufs=2,
              tail_pos=99, dma_prio=None, out_prio=None,
              merge_tail_out=False, last_osplit=4):
    f32 = mybir.dt.float32
    f16 = mybir.dt.float16
    do_pe = mode in ("full", "pe_only")
    do_act = mode == "full"
    do_out = mode in ("full", "dma_only")
    nc = bass.Bass()
    xx_d = nc.dram_tensor("xx", [ROWS_PER_CORE + 2, WPAD], f16,
                          kind="ExternalInput")
    # 3 dj blocks, each a banded lhsT [128, 128] (2 zero cols of padding)
    sm_d = nc.dram_tensor("smat", [128, 3 * 128], f16, kind="ExternalInput")
    # tail: 3 dj blocks stacked block-diag lhsT [80, 64]
    st_d = nc.dram_tensor("stail", [TAIL_STACK, 3 * TAIL_M], f16,
                          kind="ExternalInput")
    bias_in = nc.dram_tensor("bias_in", [128, 1], f32, kind="ExternalInput")
    y = nc.dram_tensor("y", [ROWS_PER_CORE, W], f16, kind="ExternalOutput")

    with TileContext(nc) as tc:
        with tc.tile_pool(name="consts", bufs=1) as cpool, \
             tc.tile_pool(name="xt", bufs=xbufs) as xpool, \
             tc.tile_pool(name="ot", bufs=obufs) as opool, \
             tc.tile_pool(name="psum", bufs=psum_bufs, space="PSUM") as ppool, \
             tc.tile_pool(name="psumt", bufs=8 - psum_bufs,
                          space="PSUM") as ppool_t:
            # const loads ride the SWDGE (gpsimd) ring so they never queue
            # ahead of tile 0's input pieces on the SP HWDGE FIFO
            s_t = cpool.tile([128, 3 * 128], f16)
            nc.gpsimd.dma_start(s_t[:], sm_d[:])
            st_t = cpool.tile([TAIL_STACK, 3 * TAIL_M], f16)
            nc.gpsimd.dma_start(st_t[:], st_d[:])
            b_t = cpool.tile([128, 1], f32)
            nc.gpsimd.dma_start(b_t[:], bias_in[:])
            zt = None
            if mode == "dma_only":
                zt = cpool.tile([128, W], f16)
                nc.gpsimd.memset(zt[:], 0.0)

            in_eng = {"sync": nc.sync, "scalar": nc.scalar,
                      "vector": nc.vector, "gpsimd": nc.gpsimd}[in_ring]
            out_eng = {"sync": nc.sync, "scalar": nc.scalar,
                       "vector": nc.vector, "gpsimd": nc.gpsimd}[out_ring]

            def copy_chunk(ci, dst_ap, src_ap, m):
                """PSUM->SBUF with bias add; alternate Act/DVE per chunk."""
                use_act = (ci % 2 == 0) if copy_split == "alt" else \
                          (copy_split == "act")
                if use_act:
                    nc.scalar.activation(
                        dst_ap, src_ap,
                        mybir.ActivationFunctionType.Identity,
                        bias=b_t[:m, :], scale=1.0,
                    )
                else:
                    nc.vector.tensor_scalar_add(dst_ap, src_ap, b_t[:m, :])

            def mm_passes(ps_list, xh, lhs_tile, mwidth, chunk_ids, cw=CHUNK):
                """3 dj passes per chunk, chunk-major."""
                for ci, c0 in enumerate(chunk_ids):
                    for dj in range(3):
                        nc.tensor.matmul(
                            ps_list[ci],
                            lhs_tile[:, dj * mwidth:(dj + 1) * mwidth],
                            xh[:, c0 + dj:c0 + dj + cw],
                            start=(dj == 0), stop=(dj == 2),
                        )

            def full_tile(t):
                k = 128
                r0 = t * TILE_OUT
                xx = xpool.tile([128, WPAD], f16, tag="xx")
                for i in range(len(xbounds) - 1):
                    lo, hi = xbounds[i], xbounds[i + 1]
                    dd = in_eng.dma_start(xx[:k, lo:hi],
                                          xx_d[r0:r0 + k, lo:hi])
                    if dma_prio is not None:
                        dd.ins.bass_priority = dma_prio
                ot = opool.tile([128, W], f16, tag="ot")
                gsz = 4                      # chunks per psum group
                for g in range(N_CHUNKS // gsz):
                    chunk_ids = [(g * gsz + i) * CHUNK for i in range(gsz)]
                    if do_pe:
                        ps_list = []
                        for _ in range(gsz):
                            ps_i = ppool.tile([128, CHUNK], f32, tag="ps")
                            ps_list.append(ps_i[:, :])
                        mm_passes(ps_list, xx, s_t, 128, chunk_ids)
                    if do_act:
                        for ci, c0 in enumerate(chunk_ids):
                            copy_chunk(g * gsz + ci,
                                       ot[:TILE_OUT, c0:c0 + CHUNK],
                                       ps_list[ci][:TILE_OUT, :], TILE_OUT)
                if do_out:
                    src_t = ot if do_act else zt
                    osp = last_osplit if t == FULL_TILES - 1 else osplit
                    ow = W // osp
                    for i in range(osp):
                        od = out_eng.dma_start(
                            y[r0:r0 + TILE_OUT, i * ow:(i + 1) * ow],
                            src_t[:TILE_OUT, i * ow:(i + 1) * ow])
                        if out_prio is not None:
                            od.ins.bass_priority = out_prio

            def tail_load():
                r0 = FULL_TILES * TILE_OUT   # shard row 504
                xxs = xpool.tile([TAIL_STACK, TAIL_GW + 2], f16, tag="txx")
                for g in range(TAIL_G):
                    gc = g * TAIL_GW
                    in_eng.dma_start(
                        xxs[g * TAIL_K:(g + 1) * TAIL_K, :],
                        xx_d[r0:r0 + TAIL_K, gc:gc + TAIL_GW + 2])
                return xxs

            def tail_tile(xxs):
                r0 = FULL_TILES * TILE_OUT   # shard row 504
                ot = opool.tile([TAIL_M, TAIL_GW], f16, tag="tot")
                if do_pe:
                    ps = ppool_t.tile([TAIL_M, CHUNK], f32, tag="tps")
                    for dj in range(3):
                        nc.tensor.matmul(
                            ps[:, :],
                            st_t[:, dj * TAIL_M:(dj + 1) * TAIL_M],
                            xxs[:, dj:dj + TAIL_GW],
                            start=(dj == 0), stop=(dj == 2),
                        )
                if do_act:
                    copy_chunk(0, ot[:, :], ps[:, :], TAIL_M)
                if do_out:
                    src_t = ot if do_act else zt
                    if merge_tail_out:
                        dst = y[r0:r0 + TAIL_ROWS, :].rearrange(
                            "r (g c) -> g r c", g=TAIL_G)
                        msrc = src_t[:TAIL_M, :TAIL_GW].rearrange(
                            "(g r) c -> g r c", g=TAIL_G)
                        out_eng.dma_start(dst, msrc)
                    else:
                        for g in range(TAIL_G):
                            out_eng.dma_start(
                                y[r0:r0 + TAIL_ROWS,
                                  g * TAIL_GW:(g + 1) * TAIL_GW],
                                src_t[g * TAIL_ROWS:(g + 1) * TAIL_ROWS,
                                      :TAIL_GW])

            def body():
                txx = tail_load()
                if tail_pos == 0:
                    tail_tile(txx)
                for t in range(FULL_TILES):
                    full_tile(t)
                    if t + 1 == tail_pos:
                        tail_tile(txx)
                if tail_pos > FULL_TILES:
                    tail_tile(txx)

            if reps == 1:
                body()
            else:
                hints = (mybir.EngineType.PE,) if hint else ()
                with tc.For_i(0, reps, 1, hint_engines=hints):
                    for _ in range(unroll):
                        body()

    _split_multi_waits(nc)
    return nc


def _make_smat(wh):
    """[128, 3*128] fp16: dj-major blocks, each a banded lhsT [128, 128]
    with band weights w[di, dj]; cols 126, 127 are zero."""
    out = np.zeros((128, 3 * 128), dtype=np.float16)
    idx = np.arange(TILE_OUT)
    for dj in range(3):
        blk = out[:, dj * 128:dj * 128 + 128]
        for di in range(3):
            blk[idx + di, idx] = wh[di, dj]
    return out


def _make_stail(wh):
    """[80, 3*64] fp16: block-diagonal stacked tail lhsT per dj."""
    out = np.zeros((TAIL_STACK, 3 * TAIL_M), dtype=np.float16)
    idx = np.arange(TAIL_ROWS)
    for dj in range(3):
        blk = out[:, dj * TAIL_M:(dj + 1) * TAIL_M]
        for g in range(TAIL_G):
            sub = blk[g * TAIL_K:(g + 1) * TAIL_K,
                      g * TAIL_ROWS:(g + 1) * TAIL_ROWS]
            for di in range(3):
                sub[idx + di, idx] = wh[di, dj]
    return out


def kernel(x, weight, bias):
    x = np.asarray(x, dtype=np.float32)
    weight = np.asarray(weight, dtype=np.float32)
    bias = np.asarray(bias, dtype=np.float32)
    wh = weight.reshape(3, 3).astype(np.float16)

    if "nc" not in _cache:
        _cache["nc"] = _build_nc()
    nc = _cache["nc"]

    xh = np.zeros((H + 2, WPAD), dtype=np.float16)
    xh[1:H + 1, 1:W + 1] = x.astype(np.float16)

    smat = _make_smat(wh)
    stail = _make_stail(wh)
    bias_bc = np.full((128, 1), bias[0], dtype=np.float32)

    in_maps = []
    for c in range(N_CORES):
        r0 = c * ROWS_PER_CORE
        in_maps.append({
            "xx": np.ascontiguousarray(xh[r0:r0 + ROWS_PER_CORE + 2, :]),
            "smat": smat,
            "stail": stail,
            "bias_in": bias_bc,
        })

    _cache["in_maps"] = in_maps
    res = None
    for attempt in range(3):
        try:
            res = bass_utils.run_bass_kernel_spmd(
                nc, in_maps, core_ids=list(range(N_CORES)))
            break
        except Exception:
            if attempt == 2:
                raise
    out = np.empty((H, W), dtype=np.float32)
    for c in range(N_CORES):
        out[c * ROWS_PER_CORE:(c + 1) * ROWS_PER_CORE, :] = \
            res.results[c]["y"].astype(np.float32)
    return out
